# revision 2
# baseline (speedup 1.0000x reference)
"""BinarizeLinear kernel for TRN2: out = x @ sign(W).

x: [32768, 512] f32, W: [512, 512] f32 -> out: [32768, 512] f32.

Data-parallel across 8 NeuronCores: each core handles 4096 tokens, W is
replicated. Per core, per iteration (4096 tokens):
  - x macro tiles (512 tokens = 1 MiB) are loaded with SWDGE
    cast-during-DMA (gpsimd), arriving in SBUF as fp16 (2^-11 rounding;
    fp32 matmuls would run at 1/4 PE rate, and sign(W) in {-1,0,1} is
    exact in fp16).
  - TensorE transposes each [128 tok, 128 din] block (PE contracts over
    the partition dim, so x must present d_in on partitions), then runs
    fp16 matmuls accumulating [128 tok, 512 dout] fp32 tiles in PSUM.
  - The kernel is PE-bound (~82k PE cycles/iter: 64k matmul + 16k
    transpose rows). Transpose-mode <-> matmul-mode switches on the PE
    cost ~160ns each (pipeline drain, measured via interleave A/B), so
    transposes and matmuls are emitted in large clumps: xt PSUM tiles
    are packed two-per-bank ([128, 2, 512] f16 = one 2KB bank), letting
    4-5 banks buffer a 64-transpose clump followed by a 64-matmul clump
    (4 mode switches/iter vs 16 for the naive per-macro order).
  - DVE drains transposed tiles PSUM->SBUF (hidden under the matmul
    clump); ScalarE (otherwise idle) casts out tiles PSUM->SBUF fp16;
    stores go out as 0.5 MiB DMAs; host concatenates shards and casts
    to fp32.
"""

import sys

if "/opt/trn_rl_repo" not in sys.path:
    sys.path.insert(0, "/opt/trn_rl_repo")

import json

import numpy as np

import concourse.bass as bass
import concourse.mybir as mybir
import concourse.tile as tile
from concourse.bass import ds
from concourse.masks import make_identity

# ---------------------------------------------------------------------------
# Workaround: the pinned walrus only accepts ONE sync wait and ONE sync
# update per instruction ("Too many sync wait commands" in setupSyncWait),
# but Tile's kernel-tail Drain carries one wait per outstanding semaphore.
# Split extras onto single-wait NoOps before (waits) / after (updates) the
# instruction — same engine, so program order preserves the semantics.
# ---------------------------------------------------------------------------

_split_uid = 0


def _split_sync(bir_json: bytes) -> bytes:
    global _split_uid
    bir = json.loads(bir_json)
    changed = False
    for fn in bir.get("functions", []):
        for blk in fn.get("blocks", []):
            insts = blk.get("instructions", [])
            out = []
            for inst in insts:
                si = inst.get("sync_info") or {}
                waits = si.get("on_wait") or []
                updates = si.get("on_update") or []
                if len(waits) > 1:
                    for w in waits[:-1]:
                        _split_uid += 1
                        out.append(
                            {
                                "name": f"I-syncsplit-w{_split_uid}",
                                "engine": inst["engine"],
                                "opcode": "NoOp",
                                "ins": [],
                                "outs": [],
                                "sync_info": {"on_update": [], "on_wait": [w]},
                            }
                        )
                    si["on_wait"] = [waits[-1]]
                    changed = True
                out.append(inst)
                if len(updates) > 1:
                    si["on_update"] = [updates[0]]
                    for u in updates[1:]:
                        _split_uid += 1
                        out.append(
                            {
                                "name": f"I-syncsplit-u{_split_uid}",
                                "engine": inst["engine"],
                                "opcode": "NoOp",
                                "ins": [],
                                "outs": [],
                                "sync_info": {"on_update": [u], "on_wait": []},
                            }
                        )
                    changed = True
            blk["instructions"] = out
    if not changed:
        return bir_json
    return json.dumps(bir).encode()


def _install_sync_split_patch() -> None:
    import concourse.bass2jax as bass2jax
    import concourse.bass_utils as bass_utils

    orig = bass_utils.compile_bir_kernel
    if getattr(orig, "_sync_split_patched", False):
        return

    def patched(bir_json, tmpdir, neff_name="file.neff", **kw):
        return orig(_split_sync(bir_json), tmpdir, neff_name, **kw)

    patched._sync_split_patched = True
    bass_utils.compile_bir_kernel = patched
    bass2jax.compile_bir_kernel = patched


_install_sync_split_patch()

N_CORES = 8
N_TOKENS = 32768
D_IN = 512
D_OUT = 512

TOK_PER_CORE = N_TOKENS // N_CORES  # 4096
P = 128  # partitions
K_CHUNKS = D_IN // P  # 4

import os as _os

MACRO = 4  # token tiles per DMA batch (512 tokens = 1 MiB f32)
# token tiles per PE transpose/matmul clump: 16 -> 64T/64M, 4 mode
# switches per 4096-token iteration
GROUP = int(_os.environ.get("GROUP_TILES", "16"))
XTPS_BUFS = int(_os.environ.get("XTPS_BUFS", "5"))
OUTPS_BUFS = int(_os.environ.get("OUTPS_BUFS", "3"))
OUT_SB_BUFS = int(_os.environ.get("OUT_SB_BUFS", "5"))
XT_BUFS = int(_os.environ.get("XT_BUFS", "5"))
XIN_BUFS = int(_os.environ.get("XIN_BUFS", "6"))

F32 = mybir.dt.float32
# fp16 for the matmul operands and the output store: sign(W) is exact in
# fp16, x ~ N(0,1) casts with 2^-11 rel error (8x better than bf16), and
# out (|.|<~150 << 65504) stores in half the bytes of f32.
F16 = mybir.dt.float16


def build_kernel(nc: bass.Bass, repeat: int = 1, macro: int | None = None) -> None:
    N_GROUP = TOK_PER_CORE // (GROUP * P)
    n_m = GROUP // MACRO  # macros per group
    x = nc.dram_tensor("x", [TOK_PER_CORE, D_IN], F32, kind="ExternalInput").ap()
    w = nc.dram_tensor("W", [D_IN, D_OUT], F32, kind="ExternalInput").ap()
    out = nc.dram_tensor("out", [TOK_PER_CORE, D_OUT], F16, kind="ExternalOutput").ap()

    # [p, a, d] view: token t = a*128 + p within a macro block of 512 tokens
    x_v = x.rearrange("(a p) d -> p a d", p=P)  # [128, 32, 512]
    out_v = out.rearrange("(a p) d -> p a d", p=P)  # [128, 32, 512]
    w_v = w.rearrange("(k p) d -> p k d", p=P)  # [128, 4, 512]

    with tile.TileContext(nc) as tc:
        with (
            tc.tile_pool(name="const", bufs=1) as const_pool,
            tc.tile_pool(name="xin", bufs=XIN_BUFS) as xin_pool,
            tc.tile_pool(name="xt", bufs=XT_BUFS) as xt_pool,
            tc.tile_pool(name="outsb", bufs=OUT_SB_BUFS) as out_pool,
            tc.tile_pool(name="xt_ps", bufs=XTPS_BUFS, space="PSUM") as xtps_pool,
            tc.tile_pool(name="out_ps", bufs=OUTPS_BUFS, space="PSUM") as outps_pool,
        ):
            # --- constants: identity for PE transpose, binarized weight ---
            ident = const_pool.tile([P, P], F16)
            make_identity(nc, ident[:])

            w_f32 = const_pool.tile([P, K_CHUNKS, D_OUT], F32)
            nc.sync.dma_start(w_f32[:], w_v[:])
            w_b = const_pool.tile([P, K_CHUNKS, D_OUT], F16)
            for k in range(K_CHUNKS):
                # sign(w): ACT LUT; +-1/0 are exact in fp16
                nc.scalar.activation(
                    w_b[:, k, :], w_f32[:, k, :], mybir.ActivationFunctionType.Sign
                )

            # --- main loop: groups of GROUP token tiles ---
            for i, g in enumerate(
                [gg for _ in range(repeat) for gg in range(N_GROUP)]
            ):
                # loads: 1 MiB macro tiles, SWDGE f32->f16 cast
                xins = []
                for m in range(n_m):
                    xin = xin_pool.tile([P, MACRO, D_IN], F16, tag="xin")
                    nc.gpsimd.dma_start(
                        xin[:], x_v[:, ds((n_m * g + m) * MACRO, MACRO), :]
                    )
                    xins.append(xin)

                # T clump: transposes, 2 token-tiles per PSUM bank
                xtsb_bufs = []
                for q in range(GROUP // 2):
                    xt_ps = xtps_pool.tile([P, 2, D_IN], F16, tag="xt_ps")
                    for ii in range(2):
                        t = 2 * q + ii
                        src = xins[t // MACRO]
                        a = t % MACRO
                        for k in range(K_CHUNKS):
                            nc.tensor.transpose(
                                xt_ps[:, ii, ds(k * P, P)],
                                src[:, a, ds(k * P, P)],
                                ident[:],
                            )
                    xt_sb = xt_pool.tile([P, 2, D_IN], F16, tag="xt_sb",
                                         name=f"xt_sb_{i}_{q}")
                    nc.vector.tensor_copy(xt_sb[:], xt_ps[:])
                    xtsb_bufs.append(xt_sb)

                # M clump: matmuls; ACT (otherwise idle) casts PSUM->SBUF f16
                out_sbs = [
                    out_pool.tile([P, MACRO, D_OUT], F16, tag="out_sb",
                                  name=f"out_sb_{i}_{m}")
                    for m in range(n_m)
                ]
                for t in range(GROUP):
                    xt_sb = xtsb_bufs[t // 2]
                    out_ps = outps_pool.tile([P, D_OUT], F32)
                    for k in range(K_CHUNKS):
                        nc.tensor.matmul(
                            out_ps[:],
                            xt_sb[:, t % 2, ds(k * P, P)],
                            w_b[:, k, :],
                            start=(k == 0),
                            stop=(k == K_CHUNKS - 1),
                        )
                    nc.scalar.activation(
                        out_sbs[t // MACRO][:, t % MACRO, :],
                        out_ps[:],
                        mybir.ActivationFunctionType.Copy,
                    )

                # stores: 0.5 MiB f16 per macro; per-tile at the very end so
                # each store departs as soon as its ACT copy lands
                last = i == repeat * N_GROUP - 1
                for m in range(n_m):
                    if last:
                        for a in range(MACRO):
                            nc.sync.dma_start(
                                out_v[:, (n_m * g + m) * MACRO + a, :],
                                out_sbs[m][:, a, :],
                            )
                    else:
                        nc.sync.dma_start(
                            out_v[:, ds((n_m * g + m) * MACRO, MACRO), :],
                            out_sbs[m][:],
                        )


def _build_nc(repeat: int = 1, macro: int | None = None) -> bass.Bass:
    nc = bass.Bass(
        "TRN2",
        target_bir_lowering=False,
        debug=False,
        num_devices=N_CORES,
    )
    build_kernel(nc, repeat=repeat, macro=macro)
    return nc


_NC_CACHE = None
_FN_CACHE = None


def _get_callable():
    """Build (once) a jitted shard_map callable over the 8 cores.

    Mirrors bass2jax.run_bass_via_pjrt's multi-core path, but cached so
    repeated kernel() calls reuse the compiled executable instead of
    re-tracing a fresh closure every time.
    """
    global _NC_CACHE, _FN_CACHE
    if _FN_CACHE is not None:
        return _FN_CACHE

    import jax
    from jax.experimental.shard_map import shard_map
    from jax.sharding import Mesh, PartitionSpec

    from concourse import bass2jax

    bass2jax.install_neuronx_cc_hook()

    if _NC_CACHE is None:
        _NC_CACHE = _build_nc()
    nc = _NC_CACHE

    partition_name = nc.partition_id_tensor.name if nc.partition_id_tensor else None
    in_names, out_names, out_avals, zero_outs = [], [], [], []
    for alloc in nc.m.functions[0].allocations:
        if not isinstance(alloc, mybir.MemoryLocationSet):
            continue
        name = alloc.memorylocations[0].name
        if alloc.kind == "ExternalInput":
            if name != partition_name:
                in_names.append(name)
        elif alloc.kind == "ExternalOutput":
            shape = tuple(alloc.tensor_shape)
            dtype = mybir.dt.np(alloc.dtype)
            out_names.append(name)
            out_avals.append(jax.core.ShapedArray(shape, dtype))
            zero_outs.append(np.zeros(shape, dtype))
    all_in_names = in_names + out_names
    if partition_name is not None:
        all_in_names = all_in_names + [partition_name]

    def _body(*args):
        operands = list(args)
        if partition_name is not None:
            operands.append(bass2jax.partition_id_tensor())
        return tuple(
            bass2jax._bass_exec_p.bind(
                *operands,
                out_avals=tuple(out_avals),
                in_names=tuple(all_in_names),
                out_names=tuple(out_names),
                lowering_input_output_aliases=(),
                sim_require_finite=True,
                sim_require_nnan=True,
                nc=nc,
            )
        )

    devices = jax.devices()[:N_CORES]
    mesh = Mesh(np.asarray(devices), ("core",))
    n_in = len(in_names) + len(out_names)
    fn = jax.jit(
        shard_map(
            _body,
            mesh=mesh,
            in_specs=(PartitionSpec("core"),) * n_in,
            out_specs=(PartitionSpec("core"),) * len(out_names),
            check_rep=False,
        ),
        keep_unused=True,
    )
    _FN_CACHE = (fn, in_names, out_names, zero_outs)
    return _FN_CACHE


def kernel(**inputs: np.ndarray) -> np.ndarray:
    x = np.ascontiguousarray(inputs["x"], dtype=np.float32)
    w = np.ascontiguousarray(inputs["W"], dtype=np.float32)
    assert x.shape == (N_TOKENS, D_IN) and w.shape == (D_IN, D_OUT)

    fn, in_names, out_names, zero_outs = _get_callable()
    per_core = {"x": np.split(x, N_CORES, axis=0), "W": [w] * N_CORES}
    concat_in = [np.concatenate(per_core[name], axis=0) for name in in_names]
    concat_in += [np.concatenate([z] * N_CORES, axis=0) for z in zero_outs]
    outs = fn(*concat_in)
    out = np.asarray(outs[out_names.index("out")])
    return out.astype(np.float32)


if __name__ == "__main__":
    rng = np.random.default_rng(0)
    x = rng.standard_normal((N_TOKENS, D_IN), dtype=np.float32)
    w = rng.standard_normal((D_IN, D_OUT), dtype=np.float32)
    got = kernel(x=x, W=w)
    want = x @ np.sign(w)
    err = np.linalg.norm(got - want) / np.linalg.norm(want)
    print("rel err:", err)
